# revision 15
# baseline (speedup 1.0000x reference)
"""Trainium2 Bass kernel for nn_InterpretableAttention (B=8, N=4096, DIM=1024).

Math: the reference returns softmax(q @ k^T, axis=-1)[:, 0, :] -- only row 0
of the attention matrix. Per batch b:
    q0       = Wq @ x[b,0] + bq                     [DIM]
    v        = Wk^T @ q0 = M @ x[b,0] + c           [DIM]
               with M = Wk^T Wq, c = Wk^T bq  (weight-only fold, host-side)
    scores_m = x[b,m] . v   (+ q0.bk, constant -> cancels in softmax)
    out[b]   = softmax(scores)                      [N]
bk never affects the output. The N x N score matrix and the full q/k
projections are never materialized.

Sharding: data-parallel over batch, one batch per NeuronCore (B == 8 cores).
M (fp16, 2 MB) is replicated; each core computes its own v on-device via 64
[128,128]x[128,1] matmuls, then streams its batch's x (fp16, host-cast,
transposed to [DIM, N]) through 64 accumulating [128,1]x[128,512] matmuls.
fp16 halves both HBM traffic and PE cycles vs fp32 (4 cyc/row -> 1).

Per-core device pipeline:
  0) ~40 tiny zero matmuls at t=0 warm the PE (HAM ramps 1.2->2.4 GHz).
  A) v = M16 @ x0 + c: 64 accumulating matmuls (M^T blocks stationary),
     DVE bias-add + fp16 cast.
  B) 8 k-slice DMAs of x^T ([128,4096] fp16, 1 MB each, 8KB/partition
     contiguous) alternating sync/scalar HWDGE queues; per slice 8
     matmuls accumulate into 8 PSUM banks ([1,512] each).
  C) online softmax: per m-tile local max (DVE) + exp/accumulate (ACT),
     then a tiny combine (global max, sum of scaled tile sums) and a
     per-tile rescale split across DVE/ACT; out DMA in two halves.
"""

import os
from contextlib import ExitStack

import numpy as np

import concourse.bass as bass  # noqa: F401
import concourse.tile as tile
from concourse import bacc, mybir
from concourse.bass_utils import run_bass_kernel_spmd

B, N, DIM = 8, 4096, 1024
P = 128          # partitions
KC = DIM // P    # 8 chunks along d
MT = 512         # m-tile (matmul moving free dim / PSUM bank)
NMT = N // MT    # 8 m-tiles
F32 = mybir.dt.float32
F16 = mybir.dt.float16
NWARM = int(os.environ.get("KERNEL_NWARM", "40"))
BARRIER = os.environ.get("KERNEL_BARRIER", "0") == "1"
RS = os.environ.get("KERNEL_RS", "0") == "1"

_program_cache = {}


def _build_program(reps: int = 1):
    key = (reps, BARRIER, RS)
    if key in _program_cache:
        return _program_cache[key]

    nc = bacc.Bacc(
        "TRN2",
        target_bir_lowering=False,
        debug=False,
        enable_asserts=False,
        num_devices=B,
    )
    # Host-prepared, per-core DRAM inputs (all partition-contiguous):
    #   xt [DIM, N] f16: x[b]^T
    #   mt [P, KC*KC*P] f16: M^T blocks, mt[p, k, j, e] = M[j*128+e, k*128+p]
    #   x0 [P, KC] f16: x0[p, c] = x[b, 0, c*128+p]
    #   ct [P, KC] f32: c[p, c'] = (Wk^T bq)[c'*128+p]
    xt = nc.dram_tensor("xt", [DIM, N], F16, kind="ExternalInput").ap()
    ct = nc.dram_tensor("ct", [P, KC], F32, kind="ExternalInput").ap()
    out = nc.dram_tensor("out", [1, N], F32, kind="ExternalOutput").ap()
    if RS:
        # TP Phase A: core j holds M^T rows of its d-chunk j (0.25 MB) and
        # all batches' x0 d-chunk j; computes partial v for ALL batches,
        # a ReduceScatter(add) sums partials and hands core b batch b's v.
        mtc = nc.dram_tensor("mtc", [P, KC * P], F16, kind="ExternalInput").ap()
        x0c = nc.dram_tensor("x0c", [P, B], F16, kind="ExternalInput").ap()
        vins = [
            nc.dram_tensor(f"vin{r}", [B * KC, P], F32).ap() for r in range(reps)
        ]
        vouts = [
            nc.dram_tensor(f"vout{r}", [KC, P], F32).ap() for r in range(reps)
        ]
    else:
        mt = nc.dram_tensor("mt", [P, KC * KC * P], F16, kind="ExternalInput").ap()
        x0 = nc.dram_tensor("x0", [P, KC], F16, kind="ExternalInput").ap()

    with tile.TileContext(nc) as tc, ExitStack() as ctx:
        singles = ctx.enter_context(tc.tile_pool(name="singles", bufs=1))
        wpool = ctx.enter_context(tc.tile_pool(name="wpool", bufs=2))
        xpool = ctx.enter_context(tc.tile_pool(name="xpool", bufs=KC))
        pspool = ctx.enter_context(tc.tile_pool(name="pspool", bufs=8, space="PSUM"))

        # ---- PE warmup: keep the PE busy from t=0 so HAM ramps to 2.4 GHz
        zt = singles.tile([P, 64], F16)
        nc.gpsimd.memset(zt, 0.0)
        wps = pspool.tile([64, 64], F32, name="pst")
        for _ in range(NWARM):
            nc.tensor.matmul(wps, zt, zt, start=True, stop=True)
        if RS:
            from concourse.masks import make_identity

            ident = singles.tile([P, P], F32)
            make_identity(nc, ident)

        for r in range(reps):
            # ---------------- Phase A: v = M @ x0 + c ----------------
            cs = wpool.tile([P, KC], F32)
            nc.sync.dma_start(cs, ct)
            if RS:
                x0s = wpool.tile([P, B], F16, name="x0s")
                nc.sync.dma_start(x0s, x0c)
                mts = wpool.tile([P, KC, P], F16, name="mts")
                nc.sync.dma_start(mts, mtc.rearrange("p (o e) -> p o e", o=KC))
                # partial v for all batches: vpp[p, o, b]
                vpp = pspool.tile([P, KC, B], F32, name="pst")
                for o in range(KC):
                    nc.tensor.matmul(
                        vpp[:, o, :], mts[:, o, :], x0s, start=True, stop=True
                    )
                # SBUF copy with (o,b)->(b,o) free permute, then PE
                # transpose so each batch's partial is 8 contiguous rows
                vA = wpool.tile([P, B * KC], F32, name="vA")
                nc.vector.tensor_copy(
                    vA.rearrange("p (b o) -> p b o", b=B),
                    vpp.rearrange("p o b -> p b o"),
                )
                vT = pspool.tile([B * KC, P], F32, name="pst")
                nc.tensor.transpose(vT, vA, ident)
                vTs = wpool.tile([B * KC, P], F32, name="vTs")
                nc.vector.tensor_copy(vTs, vT)
                nc.gpsimd.dma_start(vins[r], vTs)
                nc.gpsimd.collective_compute(
                    "ReduceScatter",
                    mybir.AluOpType.add,
                    replica_groups=[list(range(B))],
                    ins=[vins[r]],
                    outs=[vouts[r]],
                )
                vT8 = wpool.tile([KC, P], F32, name="vT8")
                nc.gpsimd.dma_start(vT8, vouts[r])
                vps = pspool.tile([P, KC], F32, name="pst")
                nc.tensor.transpose(vps, vT8, ident[:KC, :KC])
            else:
                x0s = wpool.tile([P, KC], F16, name="x0s")
                nc.sync.dma_start(x0s, x0)
                mts = wpool.tile([P, KC, KC, P], F16, name="mts")
                mtr = mt.rearrange("p (k j e) -> p k j e", k=KC, j=KC)
                nc.sync.dma_start(mts[:, : KC // 2], mtr[:, : KC // 2])
                nc.scalar.dma_start(mts[:, KC // 2 :], mtr[:, KC // 2 :])

                vps = pspool.tile([P, KC], F32, name="pst")
                for j in range(KC):
                    for k in range(KC):
                        nc.tensor.matmul(
                            vps[:, j : j + 1],
                            mts[:, k, j, :],
                            x0s[:, k : k + 1],
                            start=(k == 0),
                            stop=(k == KC - 1),
                        )
            vs16 = wpool.tile([P, KC], F16)
            nc.vector.tensor_add(vs16, vps, cs)

            # ---------------- Phase B: scores[m] = x[m] . v ----------------
            ps = []
            for t in range(NMT):
                pst = pspool.tile([1, MT], F32, name="pst")
                ps.append(pst)
            esb = singles.tile([1, N], F32)
            osb = singles.tile([1, N], F32)
            nmax = singles.tile([1, NMT], F32)
            ssum = singles.tile([1, NMT], F32)

            for k in range(KC):
                xk = xpool.tile([P, N], F16, name="xk")
                if k < KC - 1:
                    eng = nc.sync if k % 2 == 0 else nc.scalar
                    eng.dma_start(xk, xt[k * P : (k + 1) * P, :])
                    for t in range(NMT):
                        nc.tensor.matmul(
                            ps[t],
                            vs16[:, k : k + 1],
                            xk[:, t * MT : (t + 1) * MT],
                            start=(k == 0),
                            stop=False,
                        )
                else:
                    # last k-slice in 8 m-chunks: its matmuls + per-tile
                    # softmax pipeline with the DMA tail (subtile deps)
                    for t in range(NMT):
                        sl = slice(t * MT, (t + 1) * MT)
                        eng = nc.sync if t % 2 == 0 else nc.scalar
                        eng.dma_start(xk[:, sl], xt[k * P : (k + 1) * P, sl])
                        nc.tensor.matmul(
                            ps[t], vs16[:, k : k + 1], xk[:, sl],
                            start=False, stop=True,
                        )
                        # ---- Phase C (online): local max, exp, local sum
                        nc.vector.tensor_reduce(
                            nmax[:, t : t + 1],
                            ps[t],
                            axis=mybir.AxisListType.X,
                            op=mybir.AluOpType.max,
                            negate=True,
                        )
                        nc.scalar.activation(
                            esb[:, sl],
                            ps[t],
                            mybir.ActivationFunctionType.Exp,
                            bias=nmax[:, t : t + 1],
                            scale=1.0,
                            accum_out=ssum[:, t : t + 1],
                        )

            # ---- combine: g = max m_t; S = sum_t sigma_t*exp(m_t-g)
            gneg = singles.tile([1, 1], F32)
            nc.vector.tensor_reduce(
                gneg, nmax, axis=mybir.AxisListType.X, op=mybir.AluOpType.min
            )
            phi = singles.tile([1, NMT], F32)
            nc.scalar.activation(
                phi, nmax, mybir.ActivationFunctionType.Exp, bias=gneg, scale=-1.0
            )
            w8 = singles.tile([1, NMT], F32)
            nc.vector.tensor_mul(w8, phi, ssum)
            S = singles.tile([1, 1], F32)
            nc.vector.tensor_reduce(
                S, w8, axis=mybir.AxisListType.X, op=mybir.AluOpType.add
            )
            rinv = singles.tile([1, 1], F32)
            nc.vector.reciprocal(rinv, S)
            alpha = singles.tile([1, NMT], F32)
            nc.vector.tensor_scalar_mul(alpha, phi, rinv)

            # ---- rescale each tile by alpha_t (DVE/ACT split), out DMA
            # per tile-pair so the out DMAs overlap the remaining scales
            for t in range(NMT):
                sl = slice(t * MT, (t + 1) * MT)
                if t % 2 == 0:
                    nc.vector.tensor_scalar_mul(osb[:, sl], esb[:, sl], alpha[:, t : t + 1])
                else:
                    nc.scalar.mul(osb[:, sl], esb[:, sl], alpha[:, t : t + 1])
                    osl = slice((t - 1) * MT, (t + 1) * MT)
                    eng = nc.sync if (t // 2) % 2 == 0 else nc.scalar
                    eng.dma_start(out[:, osl], osb[:, osl])
            if BARRIER:
                tc.strict_bb_all_engine_barrier()

    nc.compile()
    _program_cache[key] = nc
    return nc


def _make_in_maps(x, Wq, bq, Wk):
    x = np.asarray(x, dtype=np.float32)
    wq = np.asarray(Wq, np.float32)
    wk = np.asarray(Wk, np.float32)
    bq = np.asarray(bq, np.float32)

    M = (wk.T @ wq).astype(np.float32)  # [D, D]
    c = (wk.T @ bq).astype(np.float32)  # [D]
    ct_h = np.ascontiguousarray(c.reshape(KC, P).T)  # [P, KC] f32
    x16 = x.astype(np.float16)

    in_maps = []
    if RS:
        # M blocks [o, e, j, p] = M[o*128+e, j*128+p]; core j takes its
        # d-chunk j: mtc[p, o, e] = M[o*128+e, j*128+p]
        mblk = M.reshape(KC, P, KC, P).astype(np.float16)
        x0all = x16[:, 0, :]  # [B, DIM] f16
        for j in range(B):
            mtc_h = np.ascontiguousarray(
                mblk[:, :, j, :].transpose(2, 0, 1).reshape(P, KC * P)
            )
            x0c_h = np.ascontiguousarray(x0all[:, j * P : (j + 1) * P].T)  # [P, B]
            in_maps.append(
                {
                    "xt": np.ascontiguousarray(x16[j].T),  # [DIM, N] f16
                    "mtc": mtc_h,
                    "x0c": x0c_h,
                    "ct": ct_h,
                }
            )
    else:
        # mt[p, k, j, e] = M[j*128+e, k*128+p]
        mt_h = np.ascontiguousarray(
            M.reshape(KC, P, KC, P).transpose(3, 2, 0, 1).reshape(P, KC * KC * P)
        ).astype(np.float16)
        for b in range(B):
            in_maps.append(
                {
                    "xt": np.ascontiguousarray(x16[b].T),  # [DIM, N] f16
                    "mt": mt_h,
                    "x0": np.ascontiguousarray(x16[b, 0].reshape(KC, P).T),  # [P,KC]
                    "ct": ct_h,
                }
            )
    return in_maps


def kernel(x, Wq, bq, Wk, bk):
    nc = _build_program()
    in_maps = _make_in_maps(x, Wq, bq, Wk)
    res = run_bass_kernel_spmd(nc, in_maps, core_ids=list(range(B)))
    outs = [np.asarray(res.results[b]["out"]).reshape(N) for b in range(B)]
    return np.stack(outs, axis=0).astype(np.float32)


# revision 19
# speedup vs baseline: 1.1614x; 1.1614x over previous
"""Trainium2 Bass kernel for nn_InterpretableAttention (B=8, N=4096, DIM=1024).

Math: the reference returns softmax(q @ k^T, axis=-1)[:, 0, :] -- only row 0
of the attention matrix. Per batch b:
    q0       = Wq @ x[b,0] + bq                     [DIM]
    v        = Wk^T @ q0 = M @ x[b,0] + c           [DIM]
               with M = Wk^T Wq, c = Wk^T bq  (weight-only fold, host-side)
    scores_m = x[b,m] . v   (+ q0.bk, constant -> cancels in softmax)
    out[b]   = softmax(scores)                      [N]
bk never affects the output. The N x N score matrix and the full q/k
projections are never materialized.

Sharding: data-parallel over batch, one batch per NeuronCore (B == 8 cores).
M (fp16, 2 MB) is replicated; each core computes its own v on-device via 64
[128,128]x[128,1] matmuls, then streams its batch's x (fp16, host-cast,
transposed to [DIM, N]) through 64 accumulating [128,1]x[128,512] matmuls.
fp16 halves both HBM traffic and PE cycles vs fp32 (4 cyc/row -> 1).

Per-core device pipeline:
  0) ~40 tiny zero matmuls at t=0 warm the PE (HAM ramps 1.2->2.4 GHz).
  A) v = M16 @ x0 + c: 64 accumulating matmuls (M^T blocks stationary),
     DVE bias-add + fp16 cast.
  B) 8 k-slice DMAs of x^T ([128,4096] fp16, 1 MB each, 8KB/partition
     contiguous) alternating sync/scalar HWDGE queues; per slice 8
     matmuls accumulate into 8 PSUM banks ([1,512] each).
  C) online softmax: per m-tile local max (DVE) + exp/accumulate (ACT),
     then a tiny combine (global max, sum of scaled tile sums) and a
     per-tile rescale split across DVE/ACT; out DMA in two halves.
"""

import os
from contextlib import ExitStack

import numpy as np

import concourse.bass as bass  # noqa: F401
import concourse.tile as tile
from concourse import bacc, mybir
from concourse.bass_utils import run_bass_kernel_spmd

B, N, DIM = 8, 4096, 1024
P = 128          # partitions
KC = DIM // P    # 8 chunks along d
MT = 512         # m-tile (matmul moving free dim / PSUM bank)
NMT = N // MT    # 8 m-tiles
F32 = mybir.dt.float32
F16 = mybir.dt.float16
NWARM = int(os.environ.get("KERNEL_NWARM", "40"))
BARRIER = os.environ.get("KERNEL_BARRIER", "0") == "1"
RS = os.environ.get("KERNEL_RS", "0") == "1"
XSPLIT = int(os.environ.get("KERNEL_XSPLIT", "0"))

_program_cache = {}


def _build_program(reps: int = 1):
    key = (reps, BARRIER, RS, XSPLIT)
    if key in _program_cache:
        return _program_cache[key]

    nc = bacc.Bacc(
        "TRN2",
        target_bir_lowering=False,
        debug=False,
        enable_asserts=False,
        num_devices=B,
    )
    # Host-prepared, per-core DRAM inputs (all partition-contiguous):
    #   xt [DIM, N] f16: x[b]^T
    #   mt [P, KC*KC*P] f16: M^T blocks, mt[p, k, j, e] = M[j*128+e, k*128+p]
    #   x0 [P, KC] f16: x0[p, c] = x[b, 0, c*128+p]
    #   ct [P, KC] f32: c[p, c'] = (Wk^T bq)[c'*128+p]
    xt = nc.dram_tensor("xt", [DIM, N], F16, kind="ExternalInput").ap()
    ct = nc.dram_tensor("ct", [P, KC], F32, kind="ExternalInput").ap()
    out = nc.dram_tensor("out", [1, N], F32, kind="ExternalOutput").ap()
    if RS:
        # TP Phase A: core j holds M^T rows of its d-chunk j (0.25 MB) and
        # all batches' x0 d-chunk j; computes partial v for ALL batches,
        # a ReduceScatter(add) sums partials and hands core b batch b's v.
        mtc = nc.dram_tensor("mtc", [P, KC * P], F16, kind="ExternalInput").ap()
        x0c = nc.dram_tensor("x0c", [P, B], F16, kind="ExternalInput").ap()
        vins = [
            nc.dram_tensor(f"vin{r}", [B * KC, P], F32).ap() for r in range(reps)
        ]
        vouts = [
            nc.dram_tensor(f"vout{r}", [KC, P], F32).ap() for r in range(reps)
        ]
    else:
        mt = nc.dram_tensor("mt", [P, KC * KC * P], F16, kind="ExternalInput").ap()
        x0 = nc.dram_tensor("x0", [P, KC], F16, kind="ExternalInput").ap()

    with tile.TileContext(nc) as tc, ExitStack() as ctx:
        singles = ctx.enter_context(tc.tile_pool(name="singles", bufs=1))
        wpool = ctx.enter_context(tc.tile_pool(name="wpool", bufs=2))
        xpool = ctx.enter_context(tc.tile_pool(name="xpool", bufs=KC))
        pspool = ctx.enter_context(tc.tile_pool(name="pspool", bufs=8, space="PSUM"))

        # ---- PE warmup: keep the PE busy from t=0 so HAM ramps to 2.4 GHz
        zt = singles.tile([P, 64], F16)
        nc.gpsimd.memset(zt, 0.0)
        wps = pspool.tile([64, 64], F32, name="pst")
        for _ in range(NWARM):
            nc.tensor.matmul(wps, zt, zt, start=True, stop=True)
        if RS:
            from concourse.masks import make_identity

            ident = singles.tile([P, P], F32)
            make_identity(nc, ident)

        for r in range(reps):
            # ---------------- Phase A: v = M @ x0 + c ----------------
            cs = wpool.tile([P, KC], F32)
            nc.sync.dma_start(cs, ct)
            if RS:
                x0s = wpool.tile([P, B], F16, name="x0s")
                nc.sync.dma_start(x0s, x0c)
                mts = wpool.tile([P, KC, P], F16, name="mts")
                nc.sync.dma_start(mts, mtc.rearrange("p (o e) -> p o e", o=KC))
                # partial v for all batches: vpp[p, o, b]
                vpp = pspool.tile([P, KC, B], F32, name="pst")
                for o in range(KC):
                    nc.tensor.matmul(
                        vpp[:, o, :], mts[:, o, :], x0s, start=True, stop=True
                    )
                # SBUF copy with (o,b)->(b,o) free permute, then PE
                # transpose so each batch's partial is 8 contiguous rows
                vA = wpool.tile([P, B * KC], F32, name="vA")
                nc.vector.tensor_copy(
                    vA.rearrange("p (b o) -> p b o", b=B),
                    vpp.rearrange("p o b -> p b o"),
                )
                vT = pspool.tile([B * KC, P], F32, name="pst")
                nc.tensor.transpose(vT, vA, ident)
                vTs = wpool.tile([B * KC, P], F32, name="vTs")
                nc.vector.tensor_copy(vTs, vT)
                nc.gpsimd.dma_start(vins[r], vTs)
                nc.gpsimd.collective_compute(
                    "ReduceScatter",
                    mybir.AluOpType.add,
                    replica_groups=[list(range(B))],
                    ins=[vins[r]],
                    outs=[vouts[r]],
                )
                vT8 = wpool.tile([KC, P], F32, name="vT8")
                nc.gpsimd.dma_start(vT8, vouts[r])
                vps = pspool.tile([P, KC], F32, name="pst")
                nc.tensor.transpose(vps, vT8, ident[:KC, :KC])
            else:
                x0s = wpool.tile([P, KC], F16, name="x0s")
                nc.sync.dma_start(x0s, x0)
                mts = wpool.tile([P, KC, KC, P], F16, name="mts")
                mtr = mt.rearrange("p (k j e) -> p k j e", k=KC, j=KC)
                nc.sync.dma_start(mts[:, : KC // 2], mtr[:, : KC // 2])
                nc.scalar.dma_start(mts[:, KC // 2 :], mtr[:, KC // 2 :])

                vps = pspool.tile([P, KC], F32, name="pst")
                for j in range(KC):
                    for k in range(KC):
                        nc.tensor.matmul(
                            vps[:, j : j + 1],
                            mts[:, k, j, :],
                            x0s[:, k : k + 1],
                            start=(k == 0),
                            stop=(k == KC - 1),
                        )
            vs16 = wpool.tile([P, KC], F16)
            nc.vector.tensor_add(vs16, vps, cs)

            # ---------------- Phase B: scores[m] = x[m] . v ----------------
            ps = []
            for t in range(NMT):
                pst = pspool.tile([1, MT], F32, name="pst")
                ps.append(pst)
            esb = singles.tile([1, N], F32)
            osb = singles.tile([1, N], F32)
            nmax = singles.tile([1, NMT], F32)
            ssum = singles.tile([1, NMT], F32)

            if XSPLIT == 2:
                qs = [nc.sync, nc.scalar, nc.gpsimd]
            else:
                qs = [nc.sync, nc.scalar]
            qi = 0
            for k in range(KC):
                xk = xpool.tile([P, N], F16, name="xk")
                if k < KC - 1:
                    if XSPLIT == 0:
                        qs[qi % len(qs)].dma_start(xk, xt[k * P : (k + 1) * P, :])
                        qi += 1
                    else:
                        for h in range(2):
                            sl = slice(h * (N // 2), (h + 1) * (N // 2))
                            qs[qi % len(qs)].dma_start(
                                xk[:, sl], xt[k * P : (k + 1) * P, sl]
                            )
                            qi += 1
                    for t in range(NMT):
                        nc.tensor.matmul(
                            ps[t],
                            vs16[:, k : k + 1],
                            xk[:, t * MT : (t + 1) * MT],
                            start=(k == 0),
                            stop=False,
                        )
                else:
                    # last k-slice in 8 m-chunks: its matmuls + per-tile
                    # softmax pipeline with the DMA tail (subtile deps)
                    for t in range(NMT):
                        sl = slice(t * MT, (t + 1) * MT)
                        eng = qs[qi % len(qs)]
                        qi += 1
                        eng.dma_start(xk[:, sl], xt[k * P : (k + 1) * P, sl])
                        nc.tensor.matmul(
                            ps[t], vs16[:, k : k + 1], xk[:, sl],
                            start=False, stop=True,
                        )
                        # ---- Phase C (online): local max, exp, local sum
                        nc.vector.tensor_reduce(
                            nmax[:, t : t + 1],
                            ps[t],
                            axis=mybir.AxisListType.X,
                            op=mybir.AluOpType.max,
                            negate=True,
                        )
                        nc.scalar.activation(
                            esb[:, sl],
                            ps[t],
                            mybir.ActivationFunctionType.Exp,
                            bias=nmax[:, t : t + 1],
                            scale=1.0,
                            accum_out=ssum[:, t : t + 1],
                        )

            # ---- combine: g = max m_t; S = sum_t sigma_t*exp(m_t-g)
            gneg = singles.tile([1, 1], F32)
            nc.vector.tensor_reduce(
                gneg, nmax, axis=mybir.AxisListType.X, op=mybir.AluOpType.min
            )
            phi = singles.tile([1, NMT], F32)
            nc.scalar.activation(
                phi, nmax, mybir.ActivationFunctionType.Exp, bias=gneg, scale=-1.0
            )
            w8 = singles.tile([1, NMT], F32)
            nc.vector.tensor_mul(w8, phi, ssum)
            S = singles.tile([1, 1], F32)
            nc.vector.tensor_reduce(
                S, w8, axis=mybir.AxisListType.X, op=mybir.AluOpType.add
            )
            rinv = singles.tile([1, 1], F32)
            nc.vector.reciprocal(rinv, S)
            alpha = singles.tile([1, NMT], F32)
            nc.vector.tensor_scalar_mul(alpha, phi, rinv)

            # ---- rescale each tile by alpha_t (DVE/ACT split), out DMA
            # per tile-pair so the out DMAs overlap the remaining scales
            for t in range(NMT):
                sl = slice(t * MT, (t + 1) * MT)
                if t % 2 == 0:
                    nc.vector.tensor_scalar_mul(osb[:, sl], esb[:, sl], alpha[:, t : t + 1])
                else:
                    nc.scalar.mul(osb[:, sl], esb[:, sl], alpha[:, t : t + 1])
                    osl = slice((t - 1) * MT, (t + 1) * MT)
                    eng = nc.sync if (t // 2) % 2 == 0 else nc.scalar
                    eng.dma_start(out[:, osl], osb[:, osl])
            if BARRIER:
                tc.strict_bb_all_engine_barrier()

    nc.compile()
    _program_cache[key] = nc
    return nc


def _make_in_maps(x, Wq, bq, Wk):
    x = np.asarray(x, dtype=np.float32)
    wq = np.asarray(Wq, np.float32)
    wk = np.asarray(Wk, np.float32)
    bq = np.asarray(bq, np.float32)

    M = (wk.T @ wq).astype(np.float32)  # [D, D]
    c = (wk.T @ bq).astype(np.float32)  # [D]
    ct_h = np.ascontiguousarray(c.reshape(KC, P).T)  # [P, KC] f32
    x16 = x.astype(np.float16)

    in_maps = []
    if RS:
        # M blocks [o, e, j, p] = M[o*128+e, j*128+p]; core j takes its
        # d-chunk j: mtc[p, o, e] = M[o*128+e, j*128+p]
        mblk = M.reshape(KC, P, KC, P).astype(np.float16)
        x0all = x16[:, 0, :]  # [B, DIM] f16
        for j in range(B):
            mtc_h = np.ascontiguousarray(
                mblk[:, :, j, :].transpose(2, 0, 1).reshape(P, KC * P)
            )
            x0c_h = np.ascontiguousarray(x0all[:, j * P : (j + 1) * P].T)  # [P, B]
            in_maps.append(
                {
                    "xt": np.ascontiguousarray(x16[j].T),  # [DIM, N] f16
                    "mtc": mtc_h,
                    "x0c": x0c_h,
                    "ct": ct_h,
                }
            )
    else:
        # mt[p, k, j, e] = M[j*128+e, k*128+p]
        mt_h = np.ascontiguousarray(
            M.reshape(KC, P, KC, P).transpose(3, 2, 0, 1).reshape(P, KC * KC * P)
        ).astype(np.float16)
        for b in range(B):
            in_maps.append(
                {
                    "xt": np.ascontiguousarray(x16[b].T),  # [DIM, N] f16
                    "mt": mt_h,
                    "x0": np.ascontiguousarray(x16[b, 0].reshape(KC, P).T),  # [P,KC]
                    "ct": ct_h,
                }
            )
    return in_maps


def kernel(x, Wq, bq, Wk, bk):
    nc = _build_program()
    in_maps = _make_in_maps(x, Wq, bq, Wk)
    res = run_bass_kernel_spmd(nc, in_maps, core_ids=list(range(B)))
    outs = [np.asarray(res.results[b]["out"]).reshape(N) for b in range(B)]
    return np.stack(outs, axis=0).astype(np.float32)


# revision 21
# speedup vs baseline: 1.4581x; 1.2554x over previous
"""Trainium2 Bass kernel for nn_InterpretableAttention (B=8, N=4096, DIM=1024).

Math: the reference returns softmax(q @ k^T, axis=-1)[:, 0, :] -- only row 0
of the attention matrix. Per batch b:
    q0       = Wq @ x[b,0] + bq                     [DIM]
    v        = Wk^T @ q0 = M @ x[b,0] + c           [DIM]
               with M = Wk^T Wq, c = Wk^T bq  (weight-only fold, host-side)
    scores_m = x[b,m] . v   (+ q0.bk, constant -> cancels in softmax)
    out[b]   = softmax(scores)                      [N]
bk never affects the output. The N x N score matrix and the full q/k
projections are never materialized.

Sharding: data-parallel over batch, one batch per NeuronCore (B == 8 cores).
M (fp16, 2 MB) is replicated; each core computes its own v on-device via 64
[128,128]x[128,1] matmuls, then streams its batch's x (fp16, host-cast,
transposed to [DIM, N]) through 64 accumulating [128,1]x[128,512] matmuls.
fp16 halves both HBM traffic and PE cycles vs fp32 (4 cyc/row -> 1).

Per-core device pipeline:
  0) ~40 tiny zero matmuls at t=0 warm the PE (HAM ramps 1.2->2.4 GHz).
  A) v = M16 @ x0 + c: 64 accumulating matmuls (M^T blocks stationary),
     DVE bias-add + fp16 cast.
  B) 8 k-slice DMAs of x^T ([128,4096] fp16, 1 MB each, 8KB/partition
     contiguous) alternating sync/scalar HWDGE queues; per slice 8
     matmuls accumulate into 8 PSUM banks ([1,512] each).
  C) online softmax: per m-tile local max (DVE) + exp/accumulate (ACT),
     then a tiny combine (global max, sum of scaled tile sums) and a
     per-tile rescale split across DVE/ACT; out DMA in two halves.
"""

import os
from contextlib import ExitStack

import numpy as np

import concourse.bass as bass  # noqa: F401
import concourse.tile as tile
from concourse import bacc, mybir
from concourse.bass_utils import run_bass_kernel_spmd

B, N, DIM = 8, 4096, 1024
P = 128          # partitions
KC = DIM // P    # 8 chunks along d
MT = 512         # m-tile (matmul moving free dim / PSUM bank)
NMT = N // MT    # 8 m-tiles
F32 = mybir.dt.float32
F16 = mybir.dt.float16
NWARM = int(os.environ.get("KERNEL_NWARM", "40"))
BARRIER = os.environ.get("KERNEL_BARRIER", "0") == "1"
RS = os.environ.get("KERNEL_RS", "0") == "1"
XSPLIT = int(os.environ.get("KERNEL_XSPLIT", "0"))

_program_cache = {}


def _build_program(reps: int = 1):
    key = (reps, BARRIER, RS, XSPLIT)
    if key in _program_cache:
        return _program_cache[key]

    nc = bacc.Bacc(
        "TRN2",
        target_bir_lowering=False,
        debug=False,
        enable_asserts=False,
        num_devices=B,
    )
    # Host-prepared, per-core DRAM inputs (all partition-contiguous):
    #   xt [DIM, N] f16: x[b]^T
    #   mt [P, KC*KC*P] f16: M^T blocks, mt[p, k, j, e] = M[j*128+e, k*128+p]
    #   x0 [P, KC] f16: x0[p, c] = x[b, 0, c*128+p]
    #   ct [P, KC] f32: c[p, c'] = (Wk^T bq)[c'*128+p]
    xt = nc.dram_tensor("xt", [DIM, N], F16, kind="ExternalInput").ap()
    ct = nc.dram_tensor("ct", [P, KC], F32, kind="ExternalInput").ap()
    out = nc.dram_tensor("out", [1, N], F32, kind="ExternalOutput").ap()
    if RS:
        # TP Phase A: core j holds M^T rows of its d-chunk j (0.25 MB) and
        # all batches' x0 d-chunk j; computes partial v for ALL batches,
        # a ReduceScatter(add) sums partials and hands core b batch b's v.
        mtc = nc.dram_tensor("mtc", [P, KC * P], F16, kind="ExternalInput").ap()
        x0c = nc.dram_tensor("x0c", [P, B], F16, kind="ExternalInput").ap()
        vins = [
            nc.dram_tensor(f"vin{r}", [B * KC, P], F32).ap() for r in range(reps)
        ]
        vouts = [
            nc.dram_tensor(f"vout{r}", [KC, P], F32).ap() for r in range(reps)
        ]
    else:
        mt = nc.dram_tensor("mt", [P, KC * KC * P], F16, kind="ExternalInput").ap()
        x0 = nc.dram_tensor("x0", [P, KC], F16, kind="ExternalInput").ap()

    with tile.TileContext(nc) as tc, ExitStack() as ctx:
        singles = ctx.enter_context(tc.tile_pool(name="singles", bufs=1))
        wpool = ctx.enter_context(tc.tile_pool(name="wpool", bufs=2))
        xpool = ctx.enter_context(tc.tile_pool(name="xpool", bufs=KC))
        pspool = ctx.enter_context(tc.tile_pool(name="pspool", bufs=8, space="PSUM"))

        # ---- PE warmup: keep the PE busy from t=0 so HAM ramps to 2.4 GHz
        zt = singles.tile([P, 64], F16)
        nc.gpsimd.memset(zt, 0.0)
        wps = pspool.tile([64, 64], F32, name="pst")
        for _ in range(NWARM):
            nc.tensor.matmul(wps, zt, zt, start=True, stop=True)
        if RS:
            from concourse.masks import make_identity

            ident = singles.tile([P, P], F32)
            make_identity(nc, ident)

        for r in range(reps):
            # ---------------- Phase A: v = M @ x0 + c ----------------
            cs = wpool.tile([P, KC], F32)
            nc.sync.dma_start(cs, ct)
            if RS:
                x0s = wpool.tile([P, B], F16, name="x0s")
                nc.sync.dma_start(x0s, x0c)
                mts = wpool.tile([P, KC, P], F16, name="mts")
                nc.sync.dma_start(mts, mtc.rearrange("p (o e) -> p o e", o=KC))
                # partial v for all batches: vpp[p, o, b]
                vpp = pspool.tile([P, KC, B], F32, name="pst")
                for o in range(KC):
                    nc.tensor.matmul(
                        vpp[:, o, :], mts[:, o, :], x0s, start=True, stop=True
                    )
                # SBUF copy with (o,b)->(b,o) free permute, then PE
                # transpose so each batch's partial is 8 contiguous rows
                vA = wpool.tile([P, B * KC], F32, name="vA")
                nc.vector.tensor_copy(
                    vA.rearrange("p (b o) -> p b o", b=B),
                    vpp.rearrange("p o b -> p b o"),
                )
                vT = pspool.tile([B * KC, P], F32, name="pst")
                nc.tensor.transpose(vT, vA, ident)
                vTs = wpool.tile([B * KC, P], F32, name="vTs")
                nc.vector.tensor_copy(vTs, vT)
                nc.gpsimd.dma_start(vins[r], vTs)
                nc.gpsimd.collective_compute(
                    "ReduceScatter",
                    mybir.AluOpType.add,
                    replica_groups=[list(range(B))],
                    ins=[vins[r]],
                    outs=[vouts[r]],
                )
                vT8 = wpool.tile([KC, P], F32, name="vT8")
                nc.gpsimd.dma_start(vT8, vouts[r])
                vps = pspool.tile([P, KC], F32, name="pst")
                nc.tensor.transpose(vps, vT8, ident[:KC, :KC])
            else:
                x0s = wpool.tile([P, KC], F16, name="x0s")
                nc.sync.dma_start(x0s, x0)
                mts = wpool.tile([P, KC, KC, P], F16, name="mts")
                mtr = mt.rearrange("p (k j e) -> p k j e", k=KC, j=KC)
                nc.sync.dma_start(mts[:, : KC // 2], mtr[:, : KC // 2])
                nc.scalar.dma_start(mts[:, KC // 2 :], mtr[:, KC // 2 :])

                vps = pspool.tile([P, KC], F32, name="pst")
                for j in range(KC):
                    for k in range(KC):
                        nc.tensor.matmul(
                            vps[:, j : j + 1],
                            mts[:, k, j, :],
                            x0s[:, k : k + 1],
                            start=(k == 0),
                            stop=(k == KC - 1),
                        )
            vs16 = wpool.tile([P, KC], F16)
            nc.vector.tensor_add(vs16, vps, cs)

            # ---------------- Phase B: scores[m] = x[m] . v ----------------
            ps = []
            for t in range(NMT):
                pst = pspool.tile([1, MT], F32, name="pst")
                ps.append(pst)
            esb = singles.tile([1, N], F16)
            osb = singles.tile([1, N], F32)
            nmax = singles.tile([1, NMT], F32)
            ssum = singles.tile([1, NMT], F32)

            if XSPLIT == 2:
                qs = [nc.sync, nc.scalar, nc.gpsimd]
            else:
                qs = [nc.sync, nc.scalar]
            qi = 0
            xks = []
            if XSPLIT == 3:
                # k-pairs as single 2MB DMAs (8KB runs) for k=0..5
                for a in range(3):
                    xk2 = xpool.tile([P, 2, N], F16, name="xk2", bufs=3)
                    qs[qi % len(qs)].dma_start(
                        xk2,
                        xt[2 * a * P : (2 * a + 2) * P, :].rearrange(
                            "(g p) m -> p g m", p=P
                        ),
                    )
                    qi += 1
                    xks.append(xk2[:, 0, :])
                    xks.append(xk2[:, 1, :])
            for k in range(KC):
                if XSPLIT == 3 and k < 6:
                    xk = xks[k]
                    for t in range(NMT):
                        nc.tensor.matmul(
                            ps[t],
                            vs16[:, k : k + 1],
                            xk[:, t * MT : (t + 1) * MT],
                            start=(k == 0),
                            stop=False,
                        )
                    continue
                xk = xpool.tile([P, N], F16, name="xk", bufs=2 if XSPLIT == 3 else KC)
                if k < KC - 1:
                    qs[qi % len(qs)].dma_start(xk, xt[k * P : (k + 1) * P, :])
                    qi += 1
                    for t in range(NMT):
                        nc.tensor.matmul(
                            ps[t],
                            vs16[:, k : k + 1],
                            xk[:, t * MT : (t + 1) * MT],
                            start=(k == 0),
                            stop=False,
                        )
                else:
                    # last k-slice in 8 m-chunks: its matmuls + per-tile
                    # softmax pipeline with the DMA tail (subtile deps)
                    for t in range(NMT):
                        sl = slice(t * MT, (t + 1) * MT)
                        eng = qs[qi % len(qs)]
                        qi += 1
                        eng.dma_start(xk[:, sl], xt[k * P : (k + 1) * P, sl])
                        nc.tensor.matmul(
                            ps[t], vs16[:, k : k + 1], xk[:, sl],
                            start=False, stop=True,
                        )
                        # ---- Phase C (online): local max, exp, local sum
                        nc.vector.tensor_reduce(
                            nmax[:, t : t + 1],
                            ps[t],
                            axis=mybir.AxisListType.X,
                            op=mybir.AluOpType.max,
                            negate=True,
                        )
                        nc.scalar.activation(
                            esb[:, sl],
                            ps[t],
                            mybir.ActivationFunctionType.Exp,
                            bias=nmax[:, t : t + 1],
                            scale=1.0,
                            accum_out=ssum[:, t : t + 1],
                        )

            # ---- combine: g = max m_t; S = sum_t sigma_t*exp(m_t-g)
            gneg = singles.tile([1, 1], F32)
            nc.vector.tensor_reduce(
                gneg, nmax, axis=mybir.AxisListType.X, op=mybir.AluOpType.min
            )
            phi = singles.tile([1, NMT], F32)
            nc.scalar.activation(
                phi, nmax, mybir.ActivationFunctionType.Exp, bias=gneg, scale=-1.0
            )
            w8 = singles.tile([1, NMT], F32)
            nc.vector.tensor_mul(w8, phi, ssum)
            S = singles.tile([1, 1], F32)
            nc.vector.tensor_reduce(
                S, w8, axis=mybir.AxisListType.X, op=mybir.AluOpType.add
            )
            rinv = singles.tile([1, 1], F32)
            nc.vector.reciprocal(rinv, S)
            alpha = singles.tile([1, NMT], F32)
            nc.vector.tensor_scalar_mul(alpha, phi, rinv)

            # ---- rescale each tile by alpha_t (DVE/ACT split), out DMA
            # per tile-pair so the out DMAs overlap the remaining scales
            for t in range(NMT):
                sl = slice(t * MT, (t + 1) * MT)
                if t % 2 == 0:
                    nc.vector.tensor_scalar_mul(osb[:, sl], esb[:, sl], alpha[:, t : t + 1])
                else:
                    nc.scalar.mul(osb[:, sl], esb[:, sl], alpha[:, t : t + 1])
                    osl = slice((t - 1) * MT, (t + 1) * MT)
                    eng = nc.sync if (t // 2) % 2 == 0 else nc.scalar
                    eng.dma_start(out[:, osl], osb[:, osl])
            if BARRIER:
                tc.strict_bb_all_engine_barrier()

    nc.compile()
    _program_cache[key] = nc
    return nc


def _make_in_maps(x, Wq, bq, Wk):
    x = np.asarray(x, dtype=np.float32)
    wq = np.asarray(Wq, np.float32)
    wk = np.asarray(Wk, np.float32)
    bq = np.asarray(bq, np.float32)

    M = (wk.T @ wq).astype(np.float32)  # [D, D]
    c = (wk.T @ bq).astype(np.float32)  # [D]
    ct_h = np.ascontiguousarray(c.reshape(KC, P).T)  # [P, KC] f32
    x16 = x.astype(np.float16)

    in_maps = []
    if RS:
        # M blocks [o, e, j, p] = M[o*128+e, j*128+p]; core j takes its
        # d-chunk j: mtc[p, o, e] = M[o*128+e, j*128+p]
        mblk = M.reshape(KC, P, KC, P).astype(np.float16)
        x0all = x16[:, 0, :]  # [B, DIM] f16
        for j in range(B):
            mtc_h = np.ascontiguousarray(
                mblk[:, :, j, :].transpose(2, 0, 1).reshape(P, KC * P)
            )
            x0c_h = np.ascontiguousarray(x0all[:, j * P : (j + 1) * P].T)  # [P, B]
            in_maps.append(
                {
                    "xt": np.ascontiguousarray(x16[j].T),  # [DIM, N] f16
                    "mtc": mtc_h,
                    "x0c": x0c_h,
                    "ct": ct_h,
                }
            )
    else:
        # mt[p, k, j, e] = M[j*128+e, k*128+p]
        mt_h = np.ascontiguousarray(
            M.reshape(KC, P, KC, P).transpose(3, 2, 0, 1).reshape(P, KC * KC * P)
        ).astype(np.float16)
        for b in range(B):
            in_maps.append(
                {
                    "xt": np.ascontiguousarray(x16[b].T),  # [DIM, N] f16
                    "mt": mt_h,
                    "x0": np.ascontiguousarray(x16[b, 0].reshape(KC, P).T),  # [P,KC]
                    "ct": ct_h,
                }
            )
    return in_maps


def kernel(x, Wq, bq, Wk, bk):
    nc = _build_program()
    in_maps = _make_in_maps(x, Wq, bq, Wk)
    res = run_bass_kernel_spmd(nc, in_maps, core_ids=list(range(B)))
    outs = [np.asarray(res.results[b]["out"]).reshape(N) for b in range(B)]
    return np.stack(outs, axis=0).astype(np.float32)
